# revision 49
# baseline (speedup 1.0000x reference)
"""Trainium2 Bass kernel for nn_ABCLayer (binary-basis conv layer).

Math reduction (conv is linear in its input):
    reference out = sum_n beta_n * (conv(A_n, W_eff) + sum_alpha*bias_n)
                  = conv(sum_n beta_n * A_n, W_eff) + sum_alpha * dot(beta, bias)
with A_n = sign(clip(X+v_n,0,1)-0.5) = sign(X - t_n),  t_n = 0.5 - v_n.

So the device computes ONE elementwise 3-threshold step function
    A(x) = 2*b0*[x>t0] + b1*sign(x-t1) + 2*b2*[x>t2] - (b0+b2)
followed by ONE 3x3 SAME conv (9 accumulating matmuls per output tile) and a
constant add.  W_eff / alpha (5x5 least squares on sign bases) are tiny and
folded on the host.

Distribution: pure data parallel over batch (32 images / 8 cores = 4 each).
The host shards X and pre-transposes each shard to channel-major [128, pix]
so the device needs no transposes at all; per-core outputs come back
channel-major and are un-transposed during the gather.
"""

import sys

import numpy as np

sys.path.insert(0, "/opt/trn_rl_repo")

import ml_dtypes  # noqa: E402
import concourse.bass as bass  # noqa: E402
import concourse.tile as tile  # noqa: E402
from concourse import bacc, mybir  # noqa: E402
from concourse._compat import with_exitstack  # noqa: E402
from concourse.bass_utils import run_bass_kernel_spmd  # noqa: E402

# ---------------------------------------------------------------- geometry
NCORES = 8
NB, H, WID, C = 32, 56, 56, 128        # full input NHWC
NPER = NB // NCORES                    # images per core
PIX = H * WID                            # 3136
RP, CP = H + 2, WID + 4                  # padded activation plane 58 x 60
                                       # (col pad 2 keeps bf16 writes 4B-aligned)
GR = 8                                 # output rows per PSUM group
NGRP = H // GR                         # 7 groups of 8 rows
M_FILTERS = 5

AOT = mybir.AluOpType
AFT = mybir.ActivationFunctionType
F32 = mybir.dt.float32
BF16 = mybir.dt.bfloat16


# ---------------------------------------------------------------- host math
def _prep_weights(Wf, beta, v, bias):
    """Reproduce the reference's weight preprocessing (tiny) on the host."""
    Wf = Wf.astype(np.float32)
    mean = np.float32(Wf.mean(dtype=np.float64))
    std = np.float32(np.sqrt(Wf.var(dtype=np.float64)))
    us = np.asarray(
        [-1.0 + i * 2.0 / (M_FILTERS - 1) for i in range(M_FILTERS)], np.float32
    )
    B = np.sign(Wf[None] - mean + us[:, None, None, None, None] * std).astype(
        np.float32
    )
    Bf = B.reshape(M_FILTERS, -1).T                      # [K, M]
    G = (Bf.T @ Bf).astype(np.float64)
    rhs = (Bf.T @ Wf.reshape(-1)).astype(np.float64)
    alpha = np.linalg.solve(G, rhs).astype(np.float32)   # [M]
    W_eff = np.einsum("m,mhwio->hwio", alpha, B).astype(np.float32)
    sum_alpha = float(alpha.sum(dtype=np.float64))
    cbias = sum_alpha * float(
        np.dot(beta.astype(np.float64), bias.astype(np.float64))
    )
    return W_eff, cbias


# kernel-variant knobs (A/B-tested via TimelineSim + HW bench)
DEFAULT_OPTS = dict(
    w2_engine="pool",      # "pool" | "vector"
    pad_only_memset=True,  # memset only the halo cells of apad
    out_dtype="bf16",      # "f32" | "bf16" (host upconverts)
    xin_bufs=6,
    scr_bufs=3,
    apad_bufs=8,   # two_phase keeps all chunk activation planes live
    ostage_bufs=3,
    taps=9,           # ablation: number of conv taps (9 = full conv)
    skip_elem=False,  # ablation: replace elementwise chain with one copy
    two_phase=True,   # emit all elementwise first, then all conv/evac/out
    out_dma="sync",   # "sync" | "scalar": which HWDGE ring stores outputs
    evac_split=0,     # groups per image evacuated on DVE instead of ACT
    merged_elem=True,  # 4-op DVE chain (q=stt(u0,c0,u2)) vs 5-op
    memset_engine="pool",  # "pool" | "vector"
    only_s1=False,    # ablation: A = b1*s1 + c0 (drops u0/u2/q DVE ops)
    no_sign=False,    # ablation: A = q (drops ACT sign + final stt)
    no_out_dma=False,  # ablation: skip the output store
    chunks=2,         # 1 | 2: row-chunked pipeline (finer overlap, less fill)
    u0_engine="vector",  # "vector" | "pool": who computes the u0 indicator
    inner_repeat=1,   # bench diagnostic: process the batch N times per iter
    delta_trick=True,  # emit A-c0 with pads=-c0; fold c0*colsum(W) into bias
    mtt_engine="vector",  # "vector" | "pool": engine for the u0+u2 add
    in_dma_split=False,  # alternate input DMAs across both HWDGE rings
)


# ---------------------------------------------------------------- device IR
@with_exitstack
def _emit(ctx, tc, xt, wt, bv, out, consts, repeat=1, opts=DEFAULT_OPTS):
    """Per-core program.  xt:[C, NPER,H,W] f32 in, wt:[C, 9*C] bf16 in,
    out:[C, NPER,H,W] f32 out.  repeat>1 wraps the body in a hardware loop
    (identical result, used only for wall-clock benchmarking)."""
    nc = tc.nc
    t0, t1, t2, two_b0, b1, two_b2, c0, cbias = consts
    odt = F32 if opts["out_dtype"] == "f32" else BF16

    cpool = ctx.enter_context(tc.tile_pool(name="const", bufs=1))
    xpool = ctx.enter_context(tc.tile_pool(name="xin", bufs=opts["xin_bufs"]))
    spool = ctx.enter_context(tc.tile_pool(name="scr", bufs=opts["scr_bufs"]))
    apool = ctx.enter_context(tc.tile_pool(name="apad", bufs=opts["apad_bufs"]))
    opool = ctx.enter_context(
        tc.tile_pool(name="ostage", bufs=opts["ostage_bufs"])
    )
    ppool = ctx.enter_context(
        tc.tile_pool(name="psum", bufs=8, space=bass.MemorySpace.PSUM)
    )

    wt_sb = cpool.tile([C, 9 * C], BF16)
    nc.sync.dma_start(wt_sb[:], wt[:, :])
    bias_t = cpool.tile([C, 1], F32)
    nc.sync.dma_start(bias_t[:], bv[:, :])
    nt1_t = cpool.tile([C, 1], F32)
    nc.vector.memset(nt1_t[:], -t1)
    # uniform-shift pad value: conv sees (A - c0) everywhere incl. pads; the
    # correction c0 * colsum(W_eff) rides in the per-channel bias vector bv.
    pv = -c0 if opts["delta_trick"] else 0.0

    if repeat > 1:
        loop_cm = tc.For_i(0, repeat, 1, hint_engines=(mybir.EngineType.PE,))
        ctx.enter_context(loop_cm)

    # Row-chunk descriptors.  Each chunk owns conv groups [g0, g1), an apad
    # tile spanning global padded rows [prow0, prow1), and computes the
    # elementwise activation for image rows [irow0, irow1) (chunks overlap by
    # the conv halo).  halo_top/halo_bot say which tile edge row is zero pad.
    if opts["chunks"] == 1:
        chunk_descs = [
            dict(g0=0, g1=NGRP, prow0=0, prow1=RP, irow0=0, irow1=H,
                 halo_top=True, halo_bot=True),
        ]
    else:
        chunk_descs = [
            dict(g0=0, g1=4, prow0=0, prow1=34, irow0=0, irow1=33,
                 halo_top=True, halo_bot=False),
            dict(g0=4, g1=NGRP, prow0=32, prow1=RP, irow0=31, irow1=H,
                 halo_top=False, halo_bot=True),
        ]

    def phase_a(n, ck):
        """DMA in + elementwise step function -> padded bf16 activation."""
        nrow = ck["irow1"] - ck["irow0"]          # elementwise rows
        trow = ck["prow1"] - ck["prow0"]          # apad tile rows
        xin = xpool.tile([C, nrow, WID], F32, tag="xin", name="xin")
        in_eng = nc.sync
        if opts["in_dma_split"] and (ck["g0"] > 0) != (n % 2 == 1):
            in_eng = nc.scalar
        in_eng.dma_start(xin[:], xt[:, n, ck["irow0"] : ck["irow1"], :])

        apad = apool.tile([C, trow, CP], BF16, tag="apad", name="apad")
        ms = nc.gpsimd if opts["memset_engine"] == "pool" else nc.vector
        # local interior rows (everything except zero-halo edge rows)
        li0 = ck["irow0"] + 1 - ck["prow0"]
        li1 = li0 + nrow
        if opts["pad_only_memset"]:
            if ck["halo_top"]:
                ms.memset(apad[:, 0:1, :], pv)
            if ck["halo_bot"]:
                ms.memset(apad[:, trow - 1 : trow, :], pv)
            ms.memset(apad[:, li0:li1, 0:2], pv)
            ms.memset(apad[:, li0:li1, WID + 2 : CP], pv)
        else:
            ms.memset(apad[:], pv)

        interior = apad[:, li0:li1, 2 : WID + 2]
        if opts["skip_elem"]:
            nc.vector.tensor_copy(interior, xin[:])
            return apad
        if opts["only_s1"]:
            s1 = spool.tile([C, nrow, WID], BF16, tag="s1", name="s1")
            nc.scalar.activation(s1[:], xin[:], AFT.Sign, bias=nt1_t[:, 0:1])
            nc.vector.tensor_scalar(interior, s1[:], b1, c0, AOT.mult, AOT.add)
            return apad
        u0 = spool.tile([C, nrow, WID], BF16, tag="u0", name="u0")
        u0_eng = nc.vector if opts["u0_engine"] == "vector" else nc.gpsimd
        u0_eng.tensor_scalar(u0[:], xin[:], t0, two_b0, AOT.is_gt, AOT.mult)
        u2 = spool.tile([C, nrow, WID], BF16, tag="u2", name="u2")
        nc.vector.tensor_scalar(u2[:], xin[:], t2, two_b2, AOT.is_gt, AOT.mult)
        if opts["no_sign"]:
            nc.vector.scalar_tensor_tensor(
                interior, u0[:], c0, u2[:], AOT.add, AOT.add
            )
            return apad
        s1 = spool.tile([C, nrow, WID], BF16, tag="s1", name="s1")
        nc.scalar.activation(s1[:], xin[:], AFT.Sign, bias=nt1_t[:, 0:1])
        if opts["delta_trick"]:
            # m = u0 + u2 ; A - c0 = b1*s1 + m  (c0 folded into bias vector)
            m = spool.tile([C, nrow, WID], BF16, tag="w2", name="m")
            mtt = nc.vector if opts["mtt_engine"] == "vector" else nc.gpsimd
            mtt.tensor_tensor(m[:], u0[:], u2[:], AOT.add)
            nc.vector.scalar_tensor_tensor(
                interior, s1[:], b1, m[:], AOT.mult, AOT.add
            )
        elif opts["merged_elem"]:
            # q = u0 + c0 + u2 ; A = b1*s1 + q
            q = spool.tile([C, nrow, WID], BF16, tag="w2", name="q")
            nc.vector.scalar_tensor_tensor(
                q[:], u0[:], c0, u2[:], AOT.add, AOT.add
            )
            nc.vector.scalar_tensor_tensor(
                interior, s1[:], b1, q[:], AOT.mult, AOT.add
            )
        else:
            s1c = spool.tile([C, nrow, WID], BF16, tag="s1c", name="s1c")
            nc.vector.tensor_scalar(s1c[:], s1[:], b1, c0, AOT.mult, AOT.add)
            w2 = spool.tile([C, nrow, WID], BF16, tag="w2", name="w2")
            if opts["w2_engine"] == "pool":
                nc.gpsimd.tensor_tensor(w2[:], u0[:], u2[:], AOT.add)
            else:
                nc.vector.tensor_tensor(w2[:], u0[:], u2[:], AOT.add)
            nc.vector.scalar_tensor_tensor(
                interior, s1c[:], 0.0, w2[:], AOT.add, AOT.add
            )
        return apad

    def phase_b(n, ck, apad):
        """3x3 SAME conv (9 accumulating matmuls per 8-row group) + bias +
        store."""
        ntaps = opts["taps"]
        g0, g1 = ck["g0"], ck["g1"]
        ngrp = g1 - g0
        psums = []
        for g in range(ngrp):
            psums.append(
                ppool.tile([C, GR, WID], F32, name=f"psum_g{g}", tag="opsum")
            )
        for tap in range(ntaps):
            dy, dx = divmod(tap, 3)
            lhsT = wt_sb[:, tap * C : (tap + 1) * C]
            for g in range(g0, g1):
                r = g * GR + dy - ck["prow0"]
                rhs = apad[:, r : r + GR, 1 + dx : 1 + dx + WID]
                nc.tensor.matmul(
                    psums[g - g0][:],
                    lhsT,
                    rhs,
                    start=(tap == 0),
                    stop=(tap == ntaps - 1),
                )

        orow = ngrp * GR
        ostage = opool.tile([C, orow, WID], odt, tag="ostage", name="ostage")
        for g in range(ngrp):
            dst = ostage[:, g * GR : (g + 1) * GR, :]
            if g < opts["evac_split"]:
                nc.vector.tensor_scalar(
                    dst, psums[g][:], bias_t[:, 0:1], None, AOT.add
                )
            else:
                nc.scalar.activation(
                    dst,
                    psums[g][:],
                    AFT.Identity,
                    bias=bias_t[:, 0:1],
                    scale=1.0,
                )
        if not opts["no_out_dma"]:
            dma_eng = nc.sync if opts["out_dma"] == "sync" else nc.scalar
            dma_eng.dma_start(
                out[:, n, g0 * GR : g0 * GR + orow, :], ostage[:]
            )

    work = [(n, ck) for n in range(NPER) for ck in chunk_descs]
    for _rep in range(opts["inner_repeat"]):
        if opts["two_phase"]:
            apads = [phase_a(n, ck) for (n, ck) in work]
            for (n, ck), ap_ in zip(work, apads):
                phase_b(n, ck, ap_)
        else:
            for n, ck in work:
                phase_b(n, ck, phase_a(n, ck))


def build_nc(consts, repeat=1, opts=DEFAULT_OPTS):
    nc = bacc.Bacc(
        "TRN2", target_bir_lowering=False, debug=False, enable_asserts=True
    )
    odt = F32 if opts["out_dtype"] == "f32" else BF16
    xt = nc.dram_tensor("xt", [C, NPER, H, WID], F32, kind="ExternalInput")
    wt = nc.dram_tensor("wt", [C, 9 * C], BF16, kind="ExternalInput")
    bv = nc.dram_tensor("bv", [C, 1], F32, kind="ExternalInput")
    out = nc.dram_tensor("out", [C, NPER, H, WID], odt, kind="ExternalOutput")
    with tile.TileContext(nc) as tc:
        _emit(tc, xt, wt, bv, out, consts, repeat=repeat, opts=opts)
    nc.compile()
    return nc


_NC_CACHE = {}


def _get_nc(consts):
    key = tuple(consts)
    if key not in _NC_CACHE:
        _NC_CACHE[key] = build_nc(consts)
    return _NC_CACHE[key]


def make_consts(beta, v):
    t = (0.5 - v.astype(np.float64)).astype(np.float32)
    b = beta.astype(np.float32)
    return (
        float(t[0]),
        float(t[1]),
        float(t[2]),
        float(2.0 * b[0]),
        float(b[1]),
        float(2.0 * b[2]),
        float(-b[0] - b[2]),
        0.0,  # cbias patched by caller
    )


def prepare(X, W, beta, v, bias, stride):
    """Host-side prep: weight folding, sharding, channel-major transpose.
    Returns (consts, in_maps)."""
    X = np.asarray(X, dtype=np.float32)
    Wf = np.asarray(W, dtype=np.float32)
    beta = np.asarray(beta, dtype=np.float32)
    v = np.asarray(v, dtype=np.float32)
    bias = np.asarray(bias, dtype=np.float32)
    assert int(stride) == 1, "kernel hardcodes stride=1"
    assert X.shape == (NB, H, WID, C) and Wf.shape == (3, 3, C, C)

    W_eff, cbias = _prep_weights(Wf, beta, v, bias)
    consts = list(make_consts(beta, v))
    consts[7] = float(cbias)
    consts = tuple(consts)

    # weight taps, transposed layout lhsT[tap] = W_eff[dy,dx][ci,co]
    wt = np.ascontiguousarray(
        W_eff.reshape(9, C, C).transpose(1, 0, 2).reshape(C, 9 * C)
    ).astype(ml_dtypes.bfloat16)

    # per-channel output bias: constant term + uniform-shift correction
    bvv = np.full((C, 1), cbias, np.float32)
    if DEFAULT_OPTS["delta_trick"]:
        c0 = consts[6]
        colsum = wt.astype(np.float32).reshape(C, 9 * C).sum(axis=0)
        colsum = colsum.reshape(9, C).sum(axis=0)  # [co] over taps+ci
        bvv = (bvv[:, 0] + np.float32(c0) * colsum).reshape(C, 1)
        bvv = bvv.astype(np.float32)

    in_maps = []
    for i in range(NCORES):
        xs = X[i * NPER : (i + 1) * NPER]              # [NPER,H,W,C]
        xs = np.ascontiguousarray(np.moveaxis(xs, 3, 0))  # [C,NPER,H,W]
        in_maps.append({"xt": xs, "wt": wt, "bv": bvv})
    return consts, in_maps


def kernel(X, W, beta, v, bias, stride):
    consts, in_maps = prepare(X, W, beta, v, bias, stride)

    nc = _get_nc(consts)
    res = run_bass_kernel_spmd(nc, in_maps, core_ids=list(range(NCORES)))

    outs = []
    for i in range(NCORES):
        o = np.asarray(res.results[i]["out"], dtype=np.float32)  # [C,NPER,H,W]
        outs.append(np.moveaxis(o, 0, 3))                        # [NPER,H,W,C]
    return np.concatenate(outs, axis=0)


# revision 51
# speedup vs baseline: 1.1636x; 1.1636x over previous
"""Trainium2 Bass kernel for nn_ABCLayer (binary-basis conv layer).

Math reduction (conv is linear in its input):
    reference out = sum_n beta_n * (conv(A_n, W_eff) + sum_alpha*bias_n)
                  = conv(sum_n beta_n * A_n, W_eff) + sum_alpha * dot(beta, bias)
with A_n = sign(clip(X+v_n,0,1)-0.5) = sign(X - t_n),  t_n = 0.5 - v_n.

So the device computes ONE elementwise 3-threshold step function
    A(x) = 2*b0*[x>t0] + b1*sign(x-t1) + 2*b2*[x>t2] - (b0+b2)
followed by ONE 3x3 SAME conv (9 accumulating matmuls per output tile) and a
constant add.  W_eff / alpha (5x5 least squares on sign bases) are tiny and
folded on the host.

Distribution: pure data parallel over batch (32 images / 8 cores = 4 each).
The host shards X and pre-transposes each shard to channel-major [128, pix]
so the device needs no transposes at all; per-core outputs come back
channel-major and are un-transposed during the gather.
"""

import sys

import numpy as np

sys.path.insert(0, "/opt/trn_rl_repo")

import ml_dtypes  # noqa: E402
import concourse.bass as bass  # noqa: E402
import concourse.tile as tile  # noqa: E402
from concourse import bacc, mybir  # noqa: E402
from concourse._compat import with_exitstack  # noqa: E402
from concourse.bass_utils import run_bass_kernel_spmd  # noqa: E402

# ---------------------------------------------------------------- geometry
NCORES = 8
NB, H, WID, C = 32, 56, 56, 128        # full input NHWC
NPER = NB // NCORES                    # images per core
PIX = H * WID                            # 3136
RP, CP = H + 2, WID + 4                  # padded activation plane 58 x 60
                                       # (col pad 2 keeps bf16 writes 4B-aligned)
GR = 8                                 # output rows per PSUM group
NGRP = H // GR                         # 7 groups of 8 rows
M_FILTERS = 5

AOT = mybir.AluOpType
AFT = mybir.ActivationFunctionType
F32 = mybir.dt.float32
BF16 = mybir.dt.bfloat16


# ---------------------------------------------------------------- host math
def _prep_weights(Wf, beta, v, bias):
    """Reproduce the reference's weight preprocessing (tiny) on the host."""
    Wf = Wf.astype(np.float32)
    mean = np.float32(Wf.mean(dtype=np.float64))
    std = np.float32(np.sqrt(Wf.var(dtype=np.float64)))
    us = np.asarray(
        [-1.0 + i * 2.0 / (M_FILTERS - 1) for i in range(M_FILTERS)], np.float32
    )
    B = np.sign(Wf[None] - mean + us[:, None, None, None, None] * std).astype(
        np.float32
    )
    Bf = B.reshape(M_FILTERS, -1).T                      # [K, M]
    G = (Bf.T @ Bf).astype(np.float64)
    rhs = (Bf.T @ Wf.reshape(-1)).astype(np.float64)
    alpha = np.linalg.solve(G, rhs).astype(np.float32)   # [M]
    W_eff = np.einsum("m,mhwio->hwio", alpha, B).astype(np.float32)
    sum_alpha = float(alpha.sum(dtype=np.float64))
    cbias = sum_alpha * float(
        np.dot(beta.astype(np.float64), bias.astype(np.float64))
    )
    return W_eff, cbias


# kernel-variant knobs (A/B-tested via TimelineSim + HW bench)
DEFAULT_OPTS = dict(
    w2_engine="pool",      # "pool" | "vector"
    pad_only_memset=True,  # memset only the halo cells of apad
    out_dtype="bf16",      # "f32" | "bf16" (host upconverts)
    xin_bufs=6,
    scr_bufs=3,
    apad_bufs=8,   # two_phase keeps all chunk activation planes live
    ostage_bufs=3,
    taps=9,           # ablation: number of conv taps (9 = full conv)
    skip_elem=False,  # ablation: replace elementwise chain with one copy
    two_phase=True,   # emit all elementwise first, then all conv/evac/out
    out_dma="sync",   # "sync" | "scalar": which HWDGE ring stores outputs
    evac_split=0,     # groups per image evacuated on DVE instead of ACT
    merged_elem=True,  # 4-op DVE chain (q=stt(u0,c0,u2)) vs 5-op
    memset_engine="pool",  # "pool" | "vector"
    only_s1=False,    # ablation: A = b1*s1 + c0 (drops u0/u2/q DVE ops)
    no_sign=False,    # ablation: A = q (drops ACT sign + final stt)
    no_out_dma=False,  # ablation: skip the output store
    chunks=2,         # 1 | 2: row-chunked pipeline (finer overlap, less fill)
    u0_engine="vector",  # "vector" | "pool": who computes the u0 indicator
    inner_repeat=1,   # bench diagnostic: process the batch N times per iter
    delta_trick=True,  # emit A-c0 with pads=-c0; fold c0*colsum(W) into bias
    mtt_engine="vector",  # "vector" | "pool": engine for the u0+u2 add
    in_dma_split=False,  # alternate input DMAs across both HWDGE rings
    skew=None,  # software-pipeline depth: emit B_k after A_{k+skew}.
                # None -> use two_phase flag (two_phase == skew=len(work))
)


# ---------------------------------------------------------------- device IR
@with_exitstack
def _emit(ctx, tc, xt, wt, bv, out, consts, repeat=1, opts=DEFAULT_OPTS):
    """Per-core program.  xt:[C, NPER,H,W] f32 in, wt:[C, 9*C] bf16 in,
    out:[C, NPER,H,W] f32 out.  repeat>1 wraps the body in a hardware loop
    (identical result, used only for wall-clock benchmarking)."""
    nc = tc.nc
    t0, t1, t2, two_b0, b1, two_b2, c0, cbias = consts
    odt = F32 if opts["out_dtype"] == "f32" else BF16

    cpool = ctx.enter_context(tc.tile_pool(name="const", bufs=1))
    xpool = ctx.enter_context(tc.tile_pool(name="xin", bufs=opts["xin_bufs"]))
    spool = ctx.enter_context(tc.tile_pool(name="scr", bufs=opts["scr_bufs"]))
    apool = ctx.enter_context(tc.tile_pool(name="apad", bufs=opts["apad_bufs"]))
    opool = ctx.enter_context(
        tc.tile_pool(name="ostage", bufs=opts["ostage_bufs"])
    )
    ppool = ctx.enter_context(
        tc.tile_pool(name="psum", bufs=8, space=bass.MemorySpace.PSUM)
    )

    wt_sb = cpool.tile([C, 9 * C], BF16)
    nc.sync.dma_start(wt_sb[:], wt[:, :])
    bias_t = cpool.tile([C, 1], F32)
    nc.sync.dma_start(bias_t[:], bv[:, :])
    nt1_t = cpool.tile([C, 1], F32)
    nc.vector.memset(nt1_t[:], -t1)
    # uniform-shift pad value: conv sees (A - c0) everywhere incl. pads; the
    # correction c0 * colsum(W_eff) rides in the per-channel bias vector bv.
    pv = -c0 if opts["delta_trick"] else 0.0

    if repeat > 1:
        loop_cm = tc.For_i(0, repeat, 1, hint_engines=(mybir.EngineType.PE,))
        ctx.enter_context(loop_cm)

    # Row-chunk descriptors.  Each chunk owns conv groups [g0, g1), an apad
    # tile spanning global padded rows [prow0, prow1), and computes the
    # elementwise activation for image rows [irow0, irow1) (chunks overlap by
    # the conv halo).  halo_top/halo_bot say which tile edge row is zero pad.
    if opts["chunks"] == 1:
        chunk_descs = [
            dict(g0=0, g1=NGRP, prow0=0, prow1=RP, irow0=0, irow1=H,
                 halo_top=True, halo_bot=True),
        ]
    else:
        chunk_descs = [
            dict(g0=0, g1=4, prow0=0, prow1=34, irow0=0, irow1=33,
                 halo_top=True, halo_bot=False),
            dict(g0=4, g1=NGRP, prow0=32, prow1=RP, irow0=31, irow1=H,
                 halo_top=False, halo_bot=True),
        ]

    def phase_a(n, ck):
        """DMA in + elementwise step function -> padded bf16 activation."""
        nrow = ck["irow1"] - ck["irow0"]          # elementwise rows
        trow = ck["prow1"] - ck["prow0"]          # apad tile rows
        xin = xpool.tile([C, nrow, WID], F32, tag="xin", name="xin")
        in_eng = nc.sync
        if opts["in_dma_split"] and (ck["g0"] > 0) != (n % 2 == 1):
            in_eng = nc.scalar
        in_eng.dma_start(xin[:], xt[:, n, ck["irow0"] : ck["irow1"], :])

        apad = apool.tile([C, trow, CP], BF16, tag="apad", name="apad")
        ms = nc.gpsimd if opts["memset_engine"] == "pool" else nc.vector
        # local interior rows (everything except zero-halo edge rows)
        li0 = ck["irow0"] + 1 - ck["prow0"]
        li1 = li0 + nrow
        if opts["pad_only_memset"]:
            if ck["halo_top"]:
                ms.memset(apad[:, 0:1, :], pv)
            if ck["halo_bot"]:
                ms.memset(apad[:, trow - 1 : trow, :], pv)
            ms.memset(apad[:, li0:li1, 0:2], pv)
            ms.memset(apad[:, li0:li1, WID + 2 : CP], pv)
        else:
            ms.memset(apad[:], pv)

        interior = apad[:, li0:li1, 2 : WID + 2]
        if opts["skip_elem"]:
            nc.vector.tensor_copy(interior, xin[:])
            return apad
        if opts["only_s1"]:
            s1 = spool.tile([C, nrow, WID], BF16, tag="s1", name="s1")
            nc.scalar.activation(s1[:], xin[:], AFT.Sign, bias=nt1_t[:, 0:1])
            nc.vector.tensor_scalar(interior, s1[:], b1, c0, AOT.mult, AOT.add)
            return apad
        u0 = spool.tile([C, nrow, WID], BF16, tag="u0", name="u0")
        u0_eng = nc.vector if opts["u0_engine"] == "vector" else nc.gpsimd
        u0_eng.tensor_scalar(u0[:], xin[:], t0, two_b0, AOT.is_gt, AOT.mult)
        u2 = spool.tile([C, nrow, WID], BF16, tag="u2", name="u2")
        nc.vector.tensor_scalar(u2[:], xin[:], t2, two_b2, AOT.is_gt, AOT.mult)
        if opts["no_sign"]:
            nc.vector.scalar_tensor_tensor(
                interior, u0[:], c0, u2[:], AOT.add, AOT.add
            )
            return apad
        s1 = spool.tile([C, nrow, WID], BF16, tag="s1", name="s1")
        nc.scalar.activation(s1[:], xin[:], AFT.Sign, bias=nt1_t[:, 0:1])
        if opts["delta_trick"]:
            # m = u0 + u2 ; A - c0 = b1*s1 + m  (c0 folded into bias vector)
            m = spool.tile([C, nrow, WID], BF16, tag="w2", name="m")
            mtt = nc.vector if opts["mtt_engine"] == "vector" else nc.gpsimd
            mtt.tensor_tensor(m[:], u0[:], u2[:], AOT.add)
            nc.vector.scalar_tensor_tensor(
                interior, s1[:], b1, m[:], AOT.mult, AOT.add
            )
        elif opts["merged_elem"]:
            # q = u0 + c0 + u2 ; A = b1*s1 + q
            q = spool.tile([C, nrow, WID], BF16, tag="w2", name="q")
            nc.vector.scalar_tensor_tensor(
                q[:], u0[:], c0, u2[:], AOT.add, AOT.add
            )
            nc.vector.scalar_tensor_tensor(
                interior, s1[:], b1, q[:], AOT.mult, AOT.add
            )
        else:
            s1c = spool.tile([C, nrow, WID], BF16, tag="s1c", name="s1c")
            nc.vector.tensor_scalar(s1c[:], s1[:], b1, c0, AOT.mult, AOT.add)
            w2 = spool.tile([C, nrow, WID], BF16, tag="w2", name="w2")
            if opts["w2_engine"] == "pool":
                nc.gpsimd.tensor_tensor(w2[:], u0[:], u2[:], AOT.add)
            else:
                nc.vector.tensor_tensor(w2[:], u0[:], u2[:], AOT.add)
            nc.vector.scalar_tensor_tensor(
                interior, s1c[:], 0.0, w2[:], AOT.add, AOT.add
            )
        return apad

    def phase_b(n, ck, apad):
        """3x3 SAME conv (9 accumulating matmuls per 8-row group) + bias +
        store."""
        ntaps = opts["taps"]
        g0, g1 = ck["g0"], ck["g1"]
        ngrp = g1 - g0
        psums = []
        for g in range(ngrp):
            psums.append(
                ppool.tile([C, GR, WID], F32, name=f"psum_g{g}", tag="opsum")
            )
        for tap in range(ntaps):
            dy, dx = divmod(tap, 3)
            lhsT = wt_sb[:, tap * C : (tap + 1) * C]
            for g in range(g0, g1):
                r = g * GR + dy - ck["prow0"]
                rhs = apad[:, r : r + GR, 1 + dx : 1 + dx + WID]
                nc.tensor.matmul(
                    psums[g - g0][:],
                    lhsT,
                    rhs,
                    start=(tap == 0),
                    stop=(tap == ntaps - 1),
                )

        orow = ngrp * GR
        ostage = opool.tile([C, orow, WID], odt, tag="ostage", name="ostage")
        for g in range(ngrp):
            dst = ostage[:, g * GR : (g + 1) * GR, :]
            if g < opts["evac_split"]:
                nc.vector.tensor_scalar(
                    dst, psums[g][:], bias_t[:, 0:1], None, AOT.add
                )
            else:
                nc.scalar.activation(
                    dst,
                    psums[g][:],
                    AFT.Identity,
                    bias=bias_t[:, 0:1],
                    scale=1.0,
                )
        if not opts["no_out_dma"]:
            dma_eng = nc.sync if opts["out_dma"] == "sync" else nc.scalar
            dma_eng.dma_start(
                out[:, n, g0 * GR : g0 * GR + orow, :], ostage[:]
            )

    work = [(n, ck) for n in range(NPER) for ck in chunk_descs]
    for _rep in range(opts["inner_repeat"]):
        skew = opts["skew"]
        if skew is None:
            skew = len(work) if opts["two_phase"] else 0
        apads = {}
        for i, (n, ck) in enumerate(work):
            apads[i] = phase_a(n, ck)
            j = i - skew
            if j >= 0:
                phase_b(*work[j], apads.pop(j))
        for j in range(max(0, len(work) - skew), len(work)):
            phase_b(*work[j], apads.pop(j))


def build_nc(consts, repeat=1, opts=DEFAULT_OPTS):
    nc = bacc.Bacc(
        "TRN2", target_bir_lowering=False, debug=False, enable_asserts=True
    )
    odt = F32 if opts["out_dtype"] == "f32" else BF16
    xt = nc.dram_tensor("xt", [C, NPER, H, WID], F32, kind="ExternalInput")
    wt = nc.dram_tensor("wt", [C, 9 * C], BF16, kind="ExternalInput")
    bv = nc.dram_tensor("bv", [C, 1], F32, kind="ExternalInput")
    out = nc.dram_tensor("out", [C, NPER, H, WID], odt, kind="ExternalOutput")
    with tile.TileContext(nc) as tc:
        _emit(tc, xt, wt, bv, out, consts, repeat=repeat, opts=opts)
    nc.compile()
    return nc


_NC_CACHE = {}


def _get_nc(consts):
    key = tuple(consts)
    if key not in _NC_CACHE:
        _NC_CACHE[key] = build_nc(consts)
    return _NC_CACHE[key]


def make_consts(beta, v):
    t = (0.5 - v.astype(np.float64)).astype(np.float32)
    b = beta.astype(np.float32)
    return (
        float(t[0]),
        float(t[1]),
        float(t[2]),
        float(2.0 * b[0]),
        float(b[1]),
        float(2.0 * b[2]),
        float(-b[0] - b[2]),
        0.0,  # cbias patched by caller
    )


def prepare(X, W, beta, v, bias, stride):
    """Host-side prep: weight folding, sharding, channel-major transpose.
    Returns (consts, in_maps)."""
    X = np.asarray(X, dtype=np.float32)
    Wf = np.asarray(W, dtype=np.float32)
    beta = np.asarray(beta, dtype=np.float32)
    v = np.asarray(v, dtype=np.float32)
    bias = np.asarray(bias, dtype=np.float32)
    assert int(stride) == 1, "kernel hardcodes stride=1"
    assert X.shape == (NB, H, WID, C) and Wf.shape == (3, 3, C, C)

    W_eff, cbias = _prep_weights(Wf, beta, v, bias)
    consts = list(make_consts(beta, v))
    consts[7] = float(cbias)
    consts = tuple(consts)

    # weight taps, transposed layout lhsT[tap] = W_eff[dy,dx][ci,co]
    wt = np.ascontiguousarray(
        W_eff.reshape(9, C, C).transpose(1, 0, 2).reshape(C, 9 * C)
    ).astype(ml_dtypes.bfloat16)

    # per-channel output bias: constant term + uniform-shift correction
    bvv = np.full((C, 1), cbias, np.float32)
    if DEFAULT_OPTS["delta_trick"]:
        c0 = consts[6]
        colsum = wt.astype(np.float32).reshape(C, 9 * C).sum(axis=0)
        colsum = colsum.reshape(9, C).sum(axis=0)  # [co] over taps+ci
        bvv = (bvv[:, 0] + np.float32(c0) * colsum).reshape(C, 1)
        bvv = bvv.astype(np.float32)

    in_maps = []
    for i in range(NCORES):
        xs = X[i * NPER : (i + 1) * NPER]              # [NPER,H,W,C]
        xs = np.ascontiguousarray(np.moveaxis(xs, 3, 0))  # [C,NPER,H,W]
        in_maps.append({"xt": xs, "wt": wt, "bv": bvv})
    return consts, in_maps


def kernel(X, W, beta, v, bias, stride):
    consts, in_maps = prepare(X, W, beta, v, bias, stride)

    nc = _get_nc(consts)
    res = run_bass_kernel_spmd(nc, in_maps, core_ids=list(range(NCORES)))

    outs = []
    for i in range(NCORES):
        o = np.asarray(res.results[i]["out"], dtype=np.float32)  # [C,NPER,H,W]
        outs.append(np.moveaxis(o, 0, 3))                        # [NPER,H,W,C]
    return np.concatenate(outs, axis=0)
